# revision 2
# baseline (speedup 1.0000x reference)
"""Trainium2 Bass kernel for the small actor network.

Strategy (8 NeuronCores, SPMD):
  The network is tiny; the only large tensor is w3 [256, 2048] (2 MB f32),
  so the kernel is HBM-bandwidth bound on loading w3. An on-chip AllReduce
  costs ~10us (floor) -- far more than the ~6us it would save -- so instead
  w3 is sharded by OUTPUT rows: each core loads 32 rows (256 KB), computes
  the full (tiny) front-end locally, its 32 entries of relu(w3 @ h + b3),
  and a partial of the final linear layer y_i = w4[:, rows_i] @ y3_i + b4/8.
  The host gathers by summing the 8 six-float partials (the unshard step
  for a sum-sharded output).

  Front-end: s0/s1/s5 (elementwise linears) and the three convs fold into a
  single matmul [11,128]^T @ [11,16] -> [128,16] whose last 4 contraction
  rows are rank-1 bias updates, so conv+linear+bias all land in one PSUM
  tile. h is kept in its natural [128 partitions, 16 cols] layout and w3 is
  host-permuted to match, so the big matvec is 16 PSUM-accumulating
  matmuls lhsT=[128,32], rhs=[128,1].
"""

import sys

import numpy as np

if "/opt/trn_rl_repo" not in sys.path:
    sys.path.insert(0, "/opt/trn_rl_repo")

_N_CORES = 8
_R = 32  # w3 rows per core

_nc_cache = None


def _perm():
    """Map natural on-chip layout (p, c) -> index into the reference h[2048]."""
    p = np.arange(128)
    perm = np.empty((128, 16), np.int64)
    perm[:, 0] = p                     # s0
    perm[:, 1] = 128 + p               # s1
    for t in range(5):
        perm[:, 2 + t] = 256 + 5 * p + t    # s2 (channel-major flat: c*5+t)
        perm[:, 7 + t] = 896 + 5 * p + t    # s3
    for t in range(3):
        perm[:, 12 + t] = 1536 + 3 * p + t  # s4
    perm[:, 15] = 1920 + p             # s5
    return perm


def _prep(x, conv_w, conv_b, w0, b0, w1, b1, w2, b2, w3, b3, w4, b4):
    x = np.asarray(x, np.float32).reshape(6, 8)

    # km [11, 144]: cols 0-127 = lhsT (stationary), cols 128-143 = rhs.
    # Contraction rows 0-3: conv taps; 4-6: w0/w1/w2; 7-10: bias rank-1 rows.
    km = np.zeros((11, 144), np.float32)
    km[0:4, 0:128] = np.asarray(conv_w, np.float32)[:, 0, :].T
    km[4, 0:128] = np.asarray(w0, np.float32)[:, 0]
    km[5, 0:128] = np.asarray(w1, np.float32)[:, 0]
    km[6, 0:128] = np.asarray(w2, np.float32)[:, 0]
    km[7, 0:128] = np.asarray(b0, np.float32)
    km[8, 0:128] = np.asarray(b1, np.float32)
    km[9, 0:128] = np.asarray(conv_b, np.float32)
    km[10, 0:128] = np.asarray(b2, np.float32)

    rhs = np.zeros((11, 16), np.float32)
    rhs[4, 0] = x[0, 7]
    rhs[5, 1] = x[1, 7]
    rhs[6, 15] = x[4, 7]
    for t in range(5):
        rhs[0:4, 2 + t] = x[2, t:t + 4]
        rhs[0:4, 7 + t] = x[3, t:t + 4]
    for t in range(3):
        rhs[0:4, 12 + t] = x[4, t:t + 4]
    rhs[7, 0] = 1.0
    rhs[8, 1] = 1.0
    rhs[9, 2:15] = 1.0
    rhs[10, 15] = 1.0
    km[:, 128:144] = rhs

    w3 = np.asarray(w3, np.float32)
    w4 = np.asarray(w4, np.float32)
    b3 = np.asarray(b3, np.float32)
    b4 = np.asarray(b4, np.float32)
    w3g = w3[:, _perm()]  # [256, 128, 16]

    in_maps = []
    for i in range(_N_CORES):
        rows = slice(i * _R, (i + 1) * _R)
        # wm[p, c*R + m] = w3[row0+m, perm[p, c]]
        wm = np.ascontiguousarray(
            np.transpose(w3g[rows], (1, 2, 0)).reshape(128, 16 * _R)
        )
        tail = np.zeros((_R + 1, 8), np.float32)
        tail[0:_R, 0:6] = w4[:, rows].T
        tail[0:_R, 6] = b3[rows]
        tail[_R, 0:6] = b4 / np.float32(_N_CORES)
        in_maps.append({"km": km, "wm": wm, "tail": tail})
    return in_maps


def _build_nc():
    import concourse.bass as bass
    import concourse.tile as tile
    from concourse import bacc, mybir

    f32 = mybir.dt.float32
    nc = bacc.Bacc(
        "TRN2", target_bir_lowering=False, debug=False, num_devices=_N_CORES
    )
    km_d = nc.dram_tensor("km", [11, 144], f32, kind="ExternalInput")
    wm_d = nc.dram_tensor("wm", [128, 16 * _R], f32, kind="ExternalInput")
    tail_d = nc.dram_tensor("tail", [_R + 1, 8], f32, kind="ExternalInput")
    out_d = nc.dram_tensor("out", [6, 1], f32, kind="ExternalOutput")

    relu = mybir.ActivationFunctionType.Relu

    with tile.TileContext(nc) as tc:
        with (
            tc.tile_pool(name="sb", bufs=1) as sb,
            tc.tile_pool(name="ps", bufs=1, space=bass.MemorySpace.PSUM) as ps,
        ):
            km = sb.tile([11, 144], f32)
            nc.sync.dma_start(km[:], km_d[:])
            tail = sb.tile([_R + 1, 8], f32)
            nc.sync.dma_start(tail[:], tail_d[:])
            wm = sb.tile([128, 16 * _R], f32)
            nq = 4
            q = 16 * _R // nq
            for j in range(nq):
                nc.sync.dma_start(
                    wm[:, j * q:(j + 1) * q], wm_d[:, j * q:(j + 1) * q]
                )

            # Front-end: all small layers + biases in one matmul.
            p0 = ps.tile([128, 16], f32)
            nc.tensor.matmul(
                p0[:], km[0:11, 0:128], km[0:11, 128:144], start=True, stop=True
            )
            H = sb.tile([128, 16], f32)
            nc.scalar.activation(H[:, 0:15], p0[:, 0:15], relu)
            nc.vector.tensor_copy(H[:, 15:16], p0[:, 15:16])  # s5: no relu

            # Big matvec: 16 accumulating matmuls.
            p1 = ps.tile([_R, 1], f32)
            for c in range(16):
                nc.tensor.matmul(
                    p1[:],
                    wm[:, c * _R:(c + 1) * _R],
                    H[:, c:c + 1],
                    start=(c == 0),
                    stop=(c == 15),
                )

            # y3e[0:R] = relu(p1 + b3_shard); y3e[R] = 1.0 (bias row for b4).
            y3e = sb.tile([_R + 1, 1], f32)
            nc.gpsimd.memset(y3e[_R:_R + 1, 0:1], 1.0)
            nc.scalar.activation(
                y3e[0:_R, :], p1[:], relu, bias=tail[0:_R, 6:7]
            )

            p2 = ps.tile([6, 1], f32)
            nc.tensor.matmul(
                p2[:], tail[0:_R + 1, 0:6], y3e[:], start=True, stop=True
            )
            o = sb.tile([6, 1], f32)
            nc.vector.tensor_copy(o[:], p2[:])
            nc.sync.dma_start(out_d[:], o[:])
    nc.compile()
    return nc


def run(inputs, trace=False, **kwargs):
    """Returns (output[6], BassKernelResults)."""
    from concourse.bass_utils import run_bass_kernel_spmd

    global _nc_cache
    in_maps = _prep(**{k: np.asarray(v) for k, v in inputs.items()})
    if _nc_cache is None:
        _nc_cache = _build_nc()
    res = run_bass_kernel_spmd(
        _nc_cache, in_maps, core_ids=list(range(_N_CORES)), trace=trace, **kwargs
    )
    out = np.zeros(6, np.float32)
    for r in res.results:
        out += r["out"][:, 0]
    return out.astype(np.float32), res


def kernel(**inputs):
    out, _ = run(inputs)
    return out


# revision 3
# speedup vs baseline: 1.0753x; 1.0753x over previous
"""Trainium2 Bass kernel for the small actor network.

Strategy (8 NeuronCores, SPMD):
  The network is tiny; the only large tensor is w3 [256, 2048] (2 MB f32),
  so the kernel is HBM-bandwidth bound on loading w3. An on-chip AllReduce
  costs ~10us (floor) -- far more than the ~6us it would save -- so instead
  w3 is sharded by OUTPUT rows: each core loads 32 rows (256 KB), computes
  the full (tiny) front-end locally, its 32 entries of relu(w3 @ h + b3),
  and a partial of the final linear layer y_i = w4[:, rows_i] @ y3_i + b4/8.
  The host gathers by summing the 8 six-float partials (the unshard step
  for a sum-sharded output).

  Front-end: s0/s1/s5 (elementwise linears) and the three convs fold into a
  single matmul [11,128]^T @ [11,16] -> [128,16] whose last 4 contraction
  rows are rank-1 bias updates, so conv+linear+bias all land in one PSUM
  tile. h is kept in its natural [128 partitions, 16 cols] layout and w3 is
  host-permuted to match, so the big matvec is 16 PSUM-accumulating
  matmuls lhsT=[128,32], rhs=[128,1].

  DMA: small tensors are packed into ONE buffer (per-DMA fixed cost is
  ~0.7us); wm is split in halves across the two HWDGE rings (SP + ACT
  issuing engines) so the transfers overlap. Only PE + DVE do compute
  (no ScalarE activation -> no ACT table load; no GpSimd).
"""

import sys

import numpy as np

if "/opt/trn_rl_repo" not in sys.path:
    sys.path.insert(0, "/opt/trn_rl_repo")

_N_CORES = 8
_R = 32  # w3 rows per core

_nc_cache = None


def _perm():
    """Map natural on-chip layout (p, c) -> index into the reference h[2048]."""
    p = np.arange(128)
    perm = np.empty((128, 16), np.int64)
    perm[:, 0] = p                     # s0
    perm[:, 1] = 128 + p               # s1
    for t in range(5):
        perm[:, 2 + t] = 256 + 5 * p + t    # s2 (channel-major flat: c*5+t)
        perm[:, 7 + t] = 896 + 5 * p + t    # s3
    for t in range(3):
        perm[:, 12 + t] = 1536 + 3 * p + t  # s4
    perm[:, 15] = 1920 + p             # s5
    return perm


def _prep(x, conv_w, conv_b, w0, b0, w1, b1, w2, b2, w3, b3, w4, b4):
    x = np.asarray(x, np.float32).reshape(6, 8)

    # smalls [33, 152]:
    #   [0:11, 0:128]   lhsT for the front-end matmul (conv taps, w0/w1/w2,
    #                   4 bias rank-1 rows)
    #   [0:11, 128:144] rhs (im2col windows of x, scalars, bias selectors)
    #   [0:32, 144:150] w4 shard transposed (lhsT of final matmul)
    #   [0:32, 150:151] b3 shard
    #   [0:6, 151:152]  b4 / 8
    smalls = np.zeros((33, 152), np.float32)
    smalls[0:4, 0:128] = np.asarray(conv_w, np.float32)[:, 0, :].T
    smalls[4, 0:128] = np.asarray(w0, np.float32)[:, 0]
    smalls[5, 0:128] = np.asarray(w1, np.float32)[:, 0]
    smalls[6, 0:128] = np.asarray(w2, np.float32)[:, 0]
    smalls[7, 0:128] = np.asarray(b0, np.float32)
    smalls[8, 0:128] = np.asarray(b1, np.float32)
    smalls[9, 0:128] = np.asarray(conv_b, np.float32)
    smalls[10, 0:128] = np.asarray(b2, np.float32)

    rhs = np.zeros((11, 16), np.float32)
    rhs[4, 0] = x[0, 7]
    rhs[5, 1] = x[1, 7]
    rhs[6, 15] = x[4, 7]
    for t in range(5):
        rhs[0:4, 2 + t] = x[2, t:t + 4]
        rhs[0:4, 7 + t] = x[3, t:t + 4]
    for t in range(3):
        rhs[0:4, 12 + t] = x[4, t:t + 4]
    rhs[7, 0] = 1.0
    rhs[8, 1] = 1.0
    rhs[9, 2:15] = 1.0
    rhs[10, 15] = 1.0
    smalls[0:11, 128:144] = rhs

    w3 = np.asarray(w3, np.float32)
    w4 = np.asarray(w4, np.float32)
    b3 = np.asarray(b3, np.float32)
    b4 = np.asarray(b4, np.float32)
    w3g = w3[:, _perm()]  # [256, 128, 16]

    in_maps = []
    for i in range(_N_CORES):
        rows = slice(i * _R, (i + 1) * _R)
        # wm[p, c*R + m] = w3[row0+m, perm[p, c]]
        wm = np.ascontiguousarray(
            np.transpose(w3g[rows], (1, 2, 0)).reshape(128, 16 * _R)
        )
        sm = smalls.copy()
        sm[0:_R, 144:150] = w4[:, rows].T
        sm[0:_R, 150] = b3[rows]
        sm[0:6, 151] = b4 / np.float32(_N_CORES)
        in_maps.append({"smalls": sm, "wm": wm})
    return in_maps


def _build_nc():
    import concourse.bass as bass
    import concourse.tile as tile
    from concourse import bacc, mybir

    f32 = mybir.dt.float32
    add = mybir.AluOpType.add
    amax = mybir.AluOpType.max
    nc = bacc.Bacc(
        "TRN2", target_bir_lowering=False, debug=False, num_devices=_N_CORES
    )
    sm_d = nc.dram_tensor("smalls", [33, 152], f32, kind="ExternalInput")
    wm_d = nc.dram_tensor("wm", [128, 16 * _R], f32, kind="ExternalInput")
    out_d = nc.dram_tensor("out", [6, 1], f32, kind="ExternalOutput")

    HALF = 8 * _R  # free-dim half of wm

    with tile.TileContext(nc) as tc:
        with (
            tc.tile_pool(name="sb", bufs=1) as sb,
            tc.tile_pool(name="ps", bufs=1, space=bass.MemorySpace.PSUM) as ps,
        ):
            wm = sb.tile([128, 16 * _R], f32)
            sm = sb.tile([33, 152], f32)
            # Two HWDGE rings in parallel: SP does wm half 0, ACT does
            # smalls then wm half 1.
            nc.sync.dma_start(wm[:, 0:HALF], wm_d[:, 0:HALF])
            nc.scalar.dma_start(sm[:], sm_d[:])
            nc.scalar.dma_start(wm[:, HALF:], wm_d[:, HALF:])

            # Front-end: all small layers + biases in one matmul.
            p0 = ps.tile([128, 16], f32)
            nc.tensor.matmul(
                p0[:], sm[0:11, 0:128], sm[0:11, 128:144], start=True, stop=True
            )
            H = sb.tile([128, 16], f32)
            nc.vector.tensor_scalar_max(H[:, 0:15], p0[:, 0:15], 0.0)
            nc.vector.tensor_copy(H[:, 15:16], p0[:, 15:16])  # s5: no relu

            # Big matvec: 16 accumulating matmuls.
            p1 = ps.tile([_R, 1], f32)
            for c in range(16):
                nc.tensor.matmul(
                    p1[:],
                    wm[:, c * _R:(c + 1) * _R],
                    H[:, c:c + 1],
                    start=(c == 0),
                    stop=(c == 15),
                )

            # y3 = relu(p1 + b3_shard) in one DVE op.
            y3 = sb.tile([_R, 1], f32)
            nc.vector.tensor_scalar(
                y3[:], p1[:], sm[0:_R, 150:151], 0.0, op0=add, op1=amax
            )

            p2 = ps.tile([6, 1], f32)
            nc.tensor.matmul(
                p2[:], sm[0:_R, 144:150], y3[:], start=True, stop=True
            )
            o = sb.tile([6, 1], f32)
            nc.vector.tensor_scalar_add(o[:], p2[:], sm[0:6, 151:152])
            nc.sync.dma_start(out_d[:], o[:])
    nc.compile()
    return nc


def run(inputs, trace=False, **kwargs):
    """Returns (output[6], BassKernelResults)."""
    from concourse.bass_utils import run_bass_kernel_spmd

    global _nc_cache
    in_maps = _prep(**{k: np.asarray(v) for k, v in inputs.items()})
    if _nc_cache is None:
        _nc_cache = _build_nc()
    res = run_bass_kernel_spmd(
        _nc_cache, in_maps, core_ids=list(range(_N_CORES)), trace=trace, **kwargs
    )
    out = np.zeros(6, np.float32)
    for r in res.results:
        out += r["out"][:, 0]
    return out.astype(np.float32), res


def kernel(**inputs):
    out, _ = run(inputs)
    return out
